# revision 27
# baseline (speedup 1.0000x reference)
"""Child-Sum Tree-LSTM over a complete 4-ary tree on 8 TRN2 NeuronCores.

Tree: 21845 nodes, depth 7, branching 4. Leaves (level 7) keep h=c=0, so only
the 5461 internal nodes produce output; rows 5461+ of h/c are zero.
Children of node j are 4j+1..4j+4 (contiguous), so with aligned block-sharding
per level each core's children lie in its own shard.

Distribution: levels 6,5,4,3 block-sharded across the 8 cores (512+128+32+8
= 680 nodes/core; each level's children are exactly the core's own slice of
the level below, so no communication until the top); level-3 h/c AllGathered
(128 KB fp16, 16 KB/rank); levels 2..0 (21 nodes) computed redundantly on
every core. Weights replicated.

On-device layout is transposed ([features, nodes]) so no transposes are needed
anywhere: host feeds x.T / W.T slices and transposes outputs back. Matmul
operands (x, W, U, h) and all intermediates are fp16 (full-rate TensorE at any
N, FWL weight loads, ~5e-4 element error); PSUM accumulation is fp32. Per
gate and feature chunk, one PSUM accumulation group fuses W@x (a stride-0
broadcast AP replicates the parent's x across its 4 children for the forget
gate) with U@h_sum, and a single ScalarE activation applies bias + the
nonlinearity straight out of PSUM. Child sums are single DVE tensor_reduce
ops over a [*, m, 4] view (children of node j are columns 4j..4j+3).
"""

import numpy as np

_B = 4
_H = 512
_NN = 21845
_NI = 5461
_NC = 8
_OFFS = [0, 1, 5, 21, 85, 341, 1365, 5461, 21845]

_TOP = 21                     # nodes of levels 0..2, replicated on all cores
_L3, _L4, _L5, _L6 = 8, 32, 128, 512  # per-core slice sizes of levels 3..6
_X3, _X4, _X5, _X6 = 21, 29, 61, 189  # column offsets of the l3..l6 slices
_NCOLS = _TOP + _L3 + _L4 + _L5 + _L6  # 701

_cache = {}


def _build_nc():
    import concourse.bacc as bacc
    import concourse.tile as tile
    import concourse.mybir as mybir

    F32 = mybir.dt.float32
    F16 = mybir.dt.float16
    AF = mybir.ActivationFunctionType
    AXX = mybir.AxisListType.X

    nc = bacc.Bacc("TRN2", target_bir_lowering=False)

    xT = nc.declare_dram_parameter("xT", [_H, _NCOLS], F16, isOutput=False)
    wiouT = nc.declare_dram_parameter("wiouT", [_H, 3 * _H], F16, isOutput=False)
    wfT = nc.declare_dram_parameter("wfT", [_H, _H], F16, isOutput=False)
    uiouT = nc.declare_dram_parameter("uiouT", [_H, 3 * _H], F16, isOutput=False)
    ufT = nc.declare_dram_parameter("ufT", [_H, _H], F16, isOutput=False)
    biou = nc.declare_dram_parameter("biou", [128, 12], F32, isOutput=False)
    bf = nc.declare_dram_parameter("bf", [128, 4], F32, isOutput=False)
    oh = nc.declare_dram_parameter("oh", [_H, _NCOLS], F16, isOutput=True)
    oc = nc.declare_dram_parameter("oc", [_H, _NCOLS], F16, isOutput=True)

    # collective bounce buffers: per-rank [1024, 8] fp16 rows: 512 h + 512 c
    # (feature-major); gathered to [8192, 8]
    cc_in = nc.dram_tensor("cc_in", [1024, _L3], F16)
    cc_out = nc.dram_tensor("cc_out", [_NC * 1024, _L3], F16, addr_space="Shared")

    def dview(p):
        # [F*128, n] DRAM -> [128, F, n] view (feature-chunk-major rows)
        return p.ap().rearrange("(f p) n -> p f n", p=128)

    with tile.TileContext(nc) as tc:
        with (
            tc.tile_pool(name="w", bufs=1) as wpool,
            tc.tile_pool(name="st", bufs=1) as spool,
            tc.tile_pool(name="tmp", bufs=2) as tpool,
            tc.tile_pool(name="psg", bufs=3, space="PSUM") as psg,
            tc.tile_pool(name="psu", bufs=2, space="PSUM") as psu,
        ):
            # --- weights / inputs to SBUF (k-chunk-major: [:, ck, :]) ---
            # critical path first on the sync queue: biases (tiny, needed by
            # the first l6 gate ACT), w_iou chunks, xt's l6 columns, then the
            # U-side weights (not needed until l5, ~30us in).
            b_iou = wpool.tile([128, 12], F32)
            nc.sync.dma_start(out=b_iou[:], in_=biou.ap())
            b_f = wpool.tile([128, 4], F32)
            nc.sync.dma_start(out=b_f[:], in_=bf.ap())
            w_iou = wpool.tile([128, 4, 3 * _H], F16)
            xt = wpool.tile([128, 4, _NCOLS], F16)
            xt_v = dview(xT)
            for ck in range(4):
                nc.sync.dma_start(out=w_iou[:, ck, :], in_=dview(wiouT)[:, ck, :])
            nc.sync.dma_start(out=xt[:, :, _X6:], in_=xt_v[:, :, _X6:])
            nc.sync.dma_start(out=xt[:, :, 0:_X6], in_=xt_v[:, :, 0:_X6])
            w_f = wpool.tile([128, 4, _H], F16)
            nc.sync.dma_start(out=w_f[:], in_=dview(wfT))
            u_f = wpool.tile([128, 4, _H], F16)
            nc.sync.dma_start(out=u_f[:], in_=dview(ufT))
            u_iou = wpool.tile([128, 4, 3 * _H], F16)
            nc.sync.dma_start(out=u_iou[:], in_=dview(uiouT))

            # dummy sigmoid so the ACT function-table load (~1.3us) happens
            # during the DMA phase instead of stalling the first l6 gate
            warm = tpool.tile([128, 1], F16, tag="warm", name="warm", bufs=1)
            nc.vector.memset(warm[:], 0.0)
            nc.scalar.activation(warm[:], warm[:], AF.Sigmoid)

            # --- per-level h/c stores (h fp16: feeds U matmuls) ---
            h_st, c_st = {}, {}
            for key, m in (("l6", _L6), ("l5", _L5), ("l4", _L4),
                           ("l3", _L3), ("l3f", 8 * _L3), ("l2", 16),
                           ("l1", 4), ("l0", 1)):
                h_st[key] = spool.tile([128, 4, m], F16, tag=f"h_{key}",
                                       name=f"h_{key}")
                c_st[key] = spool.tile([128, 4, m], F16, tag=f"c_{key}",
                                       name=f"c_{key}")

            # ---- level 6 (children are leaves: iou = wx only) ----
            # f-outer so each feature chunk's gates/c/h pipeline across
            # PE -> ACT -> DVE while the next chunk's matmuls run
            for f in range(4):
                gf = {}
                for g in range(3):  # 0=i 1=o 2=u
                    mt = 4 * g + f
                    ps = psg.tile([128, _L6], F32, tag="ps_g", name="ps")
                    for ck in range(4):
                        nc.tensor.matmul(ps[:],
                                         w_iou[:, ck, 128 * mt:128 * (mt + 1)],
                                         xt[:, ck, _X6:_X6 + _L6],
                                         start=(ck == 0), stop=(ck == 3))
                    gt = tpool.tile([128, _L6], F16, tag=f"g6_{g}",
                                    name=f"g6_{g}")
                    nc.scalar.activation(gt[:], ps[:],
                                         AF.Tanh if g == 2 else AF.Sigmoid,
                                         bias=b_iou[:, mt:mt + 1])
                    gf[g] = gt
                cf = c_st["l6"][:, f, :]
                nc.vector.tensor_mul(cf, gf[0][:], gf[2][:])
                tc6 = tpool.tile([128, _L6], F16, tag="tc6", name="tc6")
                nc.scalar.activation(tc6[:], cf, AF.Tanh)
                nc.vector.tensor_mul(h_st["l6"][:, f, :], gf[1][:], tc6[:])

            def level(m, xcol, key, child_h, child_c):
                """One internal level of m nodes; children in child_h/child_c
                stores with 4*m node columns. xcol: column offset of this
                level's nodes in xt."""
                h_out, c_out = h_st[key], c_st[key]
                hv = child_h[:].rearrange("p f (m k) -> p f m k", k=4)

                # forget side: psum = U_f @ h_ch + W_f @ x_parent (bcast x4);
                # prod = (psum + b_f) * c_ch, fc = sum over the 4 children
                prod = tpool.tile([128, 4, 4 * m], F16, tag="prod", name="prod")
                xb = [xt[:, ck, xcol:xcol + m].broadcast_to([128, m, 4])
                      for ck in range(4)]
                for f in range(4):
                    ps_uf = psu.tile([128, 4 * m], F32, tag="ps_f", name="ps_uf")
                    # W_f@x first: it has no dependency on the child level,
                    # so PE can fill these during the previous level's tail
                    puv = ps_uf[:].rearrange("p (m k) -> p m k", k=4)
                    for ck in range(4):
                        nc.tensor.matmul(puv,
                                         w_f[:, ck, 128 * f:128 * (f + 1)],
                                         xb[ck], start=(ck == 0), stop=False)
                    for ck in range(4):
                        nc.tensor.matmul(ps_uf[:],
                                         u_f[:, ck, 128 * f:128 * (f + 1)],
                                         child_h[:, ck, :],
                                         start=False, stop=(ck == 3))
                    nc.vector.scalar_tensor_tensor(
                        prod[:, f, :], ps_uf[:], b_f[:, f:f + 1],
                        child_c[:, f, :],
                        op0=mybir.AluOpType.add, op1=mybir.AluOpType.mult)
                pv = prod[:].rearrange("p f (m k) -> p f m k", k=4)
                fc = tpool.tile([128, 4, m], F16, tag="fc", name="fc")
                hsum = tpool.tile([128, 4, m], F16, tag="hsum", name="hsum")
                with nc.allow_low_precision("4-elt child sums"):
                    # per-chunk so each chunk's U matmuls start immediately
                    for ck in range(4):
                        nc.vector.tensor_reduce(hsum[:, ck, :], hv[:, ck],
                                                AXX, mybir.AluOpType.add)
                    nc.vector.tensor_reduce(fc[:], pv[:], AXX,
                                            mybir.AluOpType.add)

                # iou gates: psum = W @ x + U @ hsum, ACT+bias from PSUM
                # pad so the i and o halves land in different PSUM banks:
                # ACT can drain one bank while PE fills the other
                ps_io = psg.tile([128, 2, 4, m], F32, tag="ps_io", name="ps_io",
                                 bufs=1, padded_shape=[128, 2, 4, 128])
                g_io = tpool.tile([128, 2, 4, m], F16, tag="g_io", name="g_io")
                ps_u = psg.tile([128, 4, m], F32, tag="ps_u", name="ps_u",
                                bufs=1)
                g_u = tpool.tile([128, 4, m], F16, tag="g_u", name="g_u")
                for g in range(3):
                    for f in range(4):
                        mt = 4 * g + f
                        sl = ps_u[:, f, :] if g == 2 else ps_io[:, g, f, :]
                        for ck in range(4):
                            nc.tensor.matmul(
                                sl, w_iou[:, ck, 128 * mt:128 * (mt + 1)],
                                xt[:, ck, xcol:xcol + m],
                                start=(ck == 0), stop=False)
                        for ck in range(4):
                            nc.tensor.matmul(
                                sl, u_iou[:, ck, 128 * mt:128 * (mt + 1)],
                                hsum[:, ck, :], start=False, stop=(ck == 3))
                        gt = g_u[:, f, :] if g == 2 else g_io[:, g, f, :]
                        nc.scalar.activation(gt, sl,
                                             AF.Tanh if g == 2 else AF.Sigmoid,
                                             bias=b_iou[:, mt:mt + 1])

                nc.vector.tensor_mul(c_out[:], g_io[:, 0], g_u[:])
                nc.vector.tensor_add(c_out[:], c_out[:], fc[:])
                tct = tpool.tile([128, 4, m], F16, tag="tct", name="tct")
                nc.scalar.activation(tct[:], c_out[:], AF.Tanh)
                nc.vector.tensor_mul(h_out[:], g_io[:, 1], tct[:])

            # ---- levels 5, 4, 3 (sharded; l3's children are exactly the
            # core's own l4 slice) ----
            level(_L5, _X5, "l5", h_st["l6"], c_st["l6"])
            level(_L4, _X4, "l4", h_st["l5"], c_st["l5"])
            level(_L3, _X3, "l3", h_st["l4"], c_st["l4"])

            # ---- outputs for sharded levels (before the collective so
            # they overlap the AllGather instead of queueing behind it) ----
            oh_v, oc_v = dview(oh), dview(oc)
            for key, c0, m in (("l3", _X3, _L3), ("l4", _X4, _L4),
                               ("l5", _X5, _L5), ("l6", _X6, _L6)):
                nc.sync.dma_start(out=oh_v[:, :, c0:c0 + m], in_=h_st[key][:])
                nc.sync.dma_start(out=oc_v[:, :, c0:c0 + m], in_=c_st[key][:])

            # ---- AllGather level-3 h/c (fp16 payload, 16 KB/rank) ----
            cc_in_v = cc_in.ap().rearrange("(t f p) n -> t p f n", p=128, f=4)
            nc.sync.dma_start(out=cc_in_v[0], in_=h_st["l3"][:])
            nc.sync.dma_start(out=cc_in_v[1], in_=c_st["l3"][:])
            nc.gpsimd.collective_compute(
                "AllGather", mybir.AluOpType.bypass,
                ins=[cc_in.ap()], outs=[cc_out.ap()],
                replica_groups=[list(range(_NC))],
            )
            cc_out_v = cc_out.ap().rearrange(
                "(r t f p) n -> t f p r n", p=128, f=4, t=2)
            for f in range(4):
                nc.sync.dma_start(
                    out=h_st["l3f"][:, f, :].rearrange("p (r n) -> p r n", r=8),
                    in_=cc_out_v[0, f])
                nc.sync.dma_start(
                    out=c_st["l3f"][:, f, :].rearrange("p (r n) -> p r n", r=8),
                    in_=cc_out_v[1, f])

            # ---- levels 2..0 (replicated) ----
            level(16, 5, "l2", h_st["l3f"], c_st["l3f"])
            level(4, 1, "l1", h_st["l2"], c_st["l2"])
            level(1, 0, "l0", h_st["l1"], c_st["l1"])

            # ---- outputs for replicated top levels ----
            for key, c0, m in (("l0", 0, 1), ("l1", 1, 4), ("l2", 5, 16)):
                nc.sync.dma_start(out=oh_v[:, :, c0:c0 + m], in_=h_st[key][:])
                nc.sync.dma_start(out=oc_v[:, :, c0:c0 + m], in_=c_st[key][:])

    nc.compile()
    return nc


def _get_nc():
    if "nc" not in _cache:
        _cache["nc"] = _build_nc()
    return _cache["nc"]


def _core_rows(k):
    return np.concatenate([
        np.arange(0, _TOP),
        np.arange(_OFFS[3] + _L3 * k, _OFFS[3] + _L3 * (k + 1)),
        np.arange(_OFFS[4] + _L4 * k, _OFFS[4] + _L4 * (k + 1)),
        np.arange(_OFFS[5] + _L5 * k, _OFFS[5] + _L5 * (k + 1)),
        np.arange(_OFFS[6] + _L6 * k, _OFFS[6] + _L6 * (k + 1)),
    ])


def _make_runner(nc):
    """Compile the SPMD module once and return a reusable callable.

    Mirrors bass2jax.run_bass_via_pjrt's multi-core path, but caches the
    jitted executable so repeat kernel() calls skip retracing/recompiling.
    """
    import jax
    import numpy as _np
    from jax.experimental.shard_map import shard_map
    from jax.sharding import Mesh, PartitionSpec
    import concourse.mybir as mybir
    from concourse import bass2jax

    bass2jax.install_neuronx_cc_hook()

    partition_name = (nc.partition_id_tensor.name
                      if nc.partition_id_tensor else None)
    in_names, out_names, out_avals, zero_shapes = [], [], [], []
    for alloc in nc.m.functions[0].allocations:
        if not isinstance(alloc, mybir.MemoryLocationSet):
            continue
        name = alloc.memorylocations[0].name
        if alloc.kind == "ExternalInput":
            if name != partition_name:
                in_names.append(name)
        elif alloc.kind == "ExternalOutput":
            shape = tuple(alloc.tensor_shape)
            dtype = mybir.dt.np(alloc.dtype)
            out_names.append(name)
            out_avals.append(jax.core.ShapedArray(shape, dtype))
            zero_shapes.append((shape, dtype))
    n_params = len(in_names)
    all_names = in_names + out_names
    if partition_name is not None:
        all_names = all_names + [partition_name]
    donate = tuple(range(n_params, n_params + len(out_names)))

    def _body(*args):
        operands = list(args)
        if partition_name is not None:
            operands.append(bass2jax.partition_id_tensor())
        outs = bass2jax._bass_exec_p.bind(
            *operands, out_avals=tuple(out_avals), in_names=tuple(all_names),
            out_names=tuple(out_names), lowering_input_output_aliases=(),
            sim_require_finite=True, sim_require_nnan=True, nc=nc)
        return tuple(outs)

    devices = jax.devices()[:_NC]
    mesh = Mesh(_np.asarray(devices), ("core",))
    specs = (PartitionSpec("core"),) * (n_params + len(out_names))
    sharded = jax.jit(
        shard_map(_body, mesh=mesh, in_specs=specs,
                  out_specs=(PartitionSpec("core"),) * len(out_names),
                  check_rep=False),
        donate_argnums=donate, keep_unused=True)

    def run(in_maps):
        concat_in = [_np.concatenate([m[k] for m in in_maps], axis=0)
                     for k in in_names]
        zeros = [_np.zeros((_NC * s[0], *s[1:]), d) for s, d in zero_shapes]
        outs = sharded(*concat_in, *zeros)
        return [
            {name: _np.asarray(outs[i]).reshape(_NC, *out_avals[i].shape)[c]
             for i, name in enumerate(out_names)}
            for c in range(_NC)
        ]

    return run


def _get_runner():
    if "runner" not in _cache:
        _cache["runner"] = _make_runner(_get_nc())
    return _cache["runner"]


def kernel(x, children, W_iou, b_iou, W_f, b_f, U_iou, U_f):
    run = _get_runner()

    x = np.asarray(x, dtype=np.float32)
    wiouT = np.ascontiguousarray(np.asarray(W_iou, np.float32).T).astype(np.float16)
    wfT = np.ascontiguousarray(np.asarray(W_f, np.float32).T).astype(np.float16)
    uiouT = np.ascontiguousarray(np.asarray(U_iou, np.float32).T).astype(np.float16)
    ufT = np.ascontiguousarray(np.asarray(U_f, np.float32).T).astype(np.float16)
    biou_p = np.ascontiguousarray(np.asarray(b_iou, np.float32).reshape(12, 128).T)
    bf_p = np.ascontiguousarray(np.asarray(b_f, np.float32).reshape(4, 128).T)

    in_maps = []
    for k in range(_NC):
        xTk = np.ascontiguousarray(x[_core_rows(k)].T).astype(np.float16)
        in_maps.append({
            "xT": xTk, "wiouT": wiouT, "wfT": wfT, "uiouT": uiouT,
            "ufT": ufT, "biou": biou_p, "bf": bf_p,
        })

    results = run(in_maps)

    h_full = np.zeros((_NN, _H), dtype=np.float32)
    c_full = np.zeros((_NN, _H), dtype=np.float32)
    oh0 = results[0]["oh"].astype(np.float32)
    oc0 = results[0]["oc"].astype(np.float32)
    h_full[0:_TOP] = oh0[:, 0:_TOP].T
    c_full[0:_TOP] = oc0[:, 0:_TOP].T
    for k in range(_NC):
        ohk = results[k]["oh"].astype(np.float32)
        ock = results[k]["oc"].astype(np.float32)
        for off, m, c0 in ((_OFFS[3], _L3, _X3), (_OFFS[4], _L4, _X4),
                           (_OFFS[5], _L5, _X5), (_OFFS[6], _L6, _X6)):
            h_full[off + m * k: off + m * (k + 1)] = ohk[:, c0:c0 + m].T
            c_full[off + m * k: off + m * (k + 1)] = ock[:, c0:c0 + m].T
    return h_full, c_full
